# revision 1
# baseline (speedup 1.0000x reference)
"""MoNet (2x GMMConv) Trainium2 kernel — 8-core SPMD, edge-parallel by dst-node range.

Strategy:
  - Host: partition edges by destination node range (6250 nodes/core), sort by
    (dst-block, src-half), pad to uniform tile structure across the 8 cores.
  - NEFF1: each core computes the full projection table proj0 = feat @ fc_w0.T
    (fp16, 512B padded rows), then edge-parallel message passing for layer 0:
    dma_gather of src rows, Gaussian weights on ACT/DVE, segment-sum via
    one-hot matmuls accumulating in PSUM per 128-node block -> h slice.
  - Host: concat + transpose h.
  - NEFF2: same with proj1 = h @ fc_w1.T and layer-1 params -> output slice.
"""
import os
import sys

sys.path.insert(0, "/opt/trn_rl_repo")
import numpy as np

STAGE = os.environ.get("MONET_STAGE", "all")  # debug: inputs|gw|proj|gather|mm|all

N_NODES = 50000
N_EDGES = 800000
IN_F = 128
HID = 64
OUT_F = 64
DIM = 2
K = 3

NCORES = 8
NPD = N_NODES // NCORES          # 6250 nodes per device
NB = 128                         # nodes per block (= psum partition dim)
NBLK = (NPD + NB - 1) // NB      # 49 blocks; last has 106 nodes
TBL_SPLIT = 32768                # int16 gather index limit
ROW = 256                        # fp16 table row elements (512B, %256B)
REAL = K * OUT_F                 # 192 meaningful elements
SG_BLKS = 2                      # blocks per supergroup (gather granularity)
GMAX = int(os.environ.get("MONET_GMAX", "16"))  # max slots per dma_gather


def _cdiv(a, b):
    return (a + b - 1) // b


def _finish(nc):
    nc.compile()
    return nc


def _host_prep(edge_index):
    """Partition/sort/pad edges; build per-core gather structure + arrays."""
    src = np.asarray(edge_index[0]).astype(np.int64)
    dst = np.asarray(edge_index[1]).astype(np.int64)
    E = src.shape[0]

    dev = dst // NPD
    loc = dst % NPD
    blk = loc // NB
    dib = (loc % NB).astype(np.int16)        # dst index within block
    bkt = (src >= TBL_SPLIT).astype(np.int64)

    # stable sort by (dev, blk, bkt)
    gkey = (dev * NBLK + blk) * 2 + bkt
    order = np.argsort(gkey, kind="stable")
    gkey_s = gkey[order]

    counts = np.bincount(gkey, minlength=NCORES * NBLK * 2).reshape(NCORES, NBLK, 2)
    tiles = np.ceil(counts.max(axis=0) / 128).astype(np.int64)  # [NBLK, 2]

    # slot layout: section-major (all lo-bucket runs first, then all hi) so
    # lo gathers only depend on the lo half of the projection table
    slot_of = np.zeros((NBLK, 2), np.int64)
    gathers = []  # (bkt, slot_start, nslots)
    slot_blk = []  # slot -> blk
    s = 0
    for b_ in (0, 1):
        for sg0 in range(0, NBLK, SG_BLKS):
            sg = range(sg0, min(sg0 + SG_BLKS, NBLK))
            run0 = s
            for b in sg:
                slot_of[b, b_] = s
                s += tiles[b, b_]
                slot_blk += [b] * tiles[b, b_]
            r = run0
            while r < s:
                n = min(GMAX, s - r)
                gathers.append((b_, r, n))
                r += n
    S = s

    # per-edge destination position in the padded slot layout
    # rank within (dev, blk, bkt) group:
    starts = np.zeros(E, np.int64)
    grp_start = np.r_[0, np.flatnonzero(np.diff(gkey_s)) + 1]
    sizes = np.diff(np.r_[grp_start, E])
    j = np.arange(E) - np.repeat(grp_start, sizes)
    blk_s = blk[order]
    bkt_s = bkt[order]
    dev_s = dev[order]
    pos = slot_of[blk_s, bkt_s] * 128 + j

    idx16 = np.zeros((NCORES, 16, S * 8), np.int16)
    dstl = np.full((NCORES, 128, S), -1, np.int16)
    psa = np.zeros((NCORES, 128, S, 2), np.float32)

    rel = (src[order] - bkt_s * TBL_SPLIT).astype(np.int16)
    idx16[dev_s, pos % 16, pos // 16] = rel
    dstl[dev_s, pos % 128, pos // 128] = dib[order]
    # pseudo filled later (by caller) using `order`/`pos`/`dev_s`
    return dict(
        tiles=tiles, gathers=gathers, slot_blk=np.array(slot_blk), S=S,
        order=order, pos=pos, dev_s=dev_s,
        idx16=idx16, dstl=dstl, psa=psa,
    )


def _build_neff(layer, S, gathers, slot_blk, tiles, scal):
    """Build one layer's Bacc program (same program for all 8 cores)."""
    import concourse.bacc as bacc
    import concourse.tile as tile
    from concourse import mybir

    f32 = mybir.dt.float32
    f16 = mybir.dt.float16
    AT = mybir.AluOpType
    ACT = mybir.ActivationFunctionType

    CDIM = IN_F if layer == 0 else HID      # proj contraction dim
    OUTD = HID if layer == 0 else OUT_F     # = 64 both layers

    nc = bacc.Bacc("TRN2", target_bir_lowering=False, debug=False, num_swdge_queues=4)
    xT = nc.declare_dram_parameter("xT", [CDIM, N_NODES], f16, isOutput=False)
    wT_in = nc.declare_dram_parameter("wT", [CDIM, REAL], f16, isOutput=False)
    idx_in = nc.declare_dram_parameter("idx", [128, S * 8], mybir.dt.int16, isOutput=False)
    dstl_in = nc.declare_dram_parameter("dstl", [128, S], mybir.dt.int16, isOutput=False)
    ps_in = nc.declare_dram_parameter("ps", [128, S, 2], f32, isOutput=False)
    bias_in = nc.declare_dram_parameter("bias", [128, OUTD], f32, isOutput=False)
    out = nc.declare_dram_parameter("out", [NPD, OUTD], f32, isOutput=True)
    tbl = nc.dram_tensor("tbl", [N_NODES, ROW], f16)

    n_ptile = _cdiv(N_NODES, 128)

    with tile.TileContext(nc) as tc:
        with (
            tc.tile_pool(name="io", bufs=1) as io,
            tc.tile_pool(name="wk", bufs=1) as wk,
            tc.tile_pool(name="proj", bufs=6) as pj,
            tc.tile_pool(name="gp", bufs=6) as gp,
            tc.tile_pool(name="ev", bufs=8) as ev,
            tc.tile_pool(name="ps", bufs=8, space="PSUM") as pp,
        ):
            # ---- static inputs ----
            idx_sb = io.tile([128, S * 8], mybir.dt.int16, name="idx_sb")
            dstl_sb = io.tile([128, S], mybir.dt.int16, name="dstl_sb")
            ps_sb = io.tile([128, S, 2], f32, name="ps_sb")
            bias_sb = io.tile([128, OUTD], f32, name="bias_sb")
            w_sb = io.tile([CDIM, REAL], f16, name="w_sb")
            iota_sb = io.tile([128, 128], mybir.dt.int16, name="iota_sb")
            gw_sb = io.tile([128, S, K], f16, name="gw_sb")
            nc.sync.dma_start(idx_sb[:], idx_in[:])
            nc.sync.dma_start(dstl_sb[:], dstl_in[:])
            nc.sync.dma_start(ps_sb[:, :, :], ps_in[:, :, :])
            nc.sync.dma_start(bias_sb[:], bias_in[:])
            nc.sync.dma_start(w_sb[:], wT_in[:])
            nc.gpsimd.iota(iota_sb[:], pattern=[[1, 128]], base=0,
                           channel_multiplier=0)

            stages = ["inputs", "gw", "proj", "gather", "mm", "all"]
            lvl = stages.index(STAGE)
            # ---- gaussian weights: gw[e,k] = exp(-.5*sum_d((p_d-mu)*isig)^2) ----
            ppw, ppb, mu, isig = scal["ppw"], scal["ppb"], scal["mu"], scal["isig"]
            p0 = wk.tile([128, S], f32, name="p0", tag="gwsc0")
            p1 = wk.tile([128, S], f32, name="p1", tag="gwsc1")
            t0 = wk.tile([128, S], f32, name="t0", tag="gwsc2")
            t1 = wk.tile([128, S], f32, name="t1", tag="gwsc3")
            for d, pd in (((0, p0), (1, p1)) if lvl >= 1 else ()):
                nc.vector.tensor_scalar(t0[:], ps_sb[:, :, 1], float(ppw[d, 1]), None, AT.mult)
                nc.vector.scalar_tensor_tensor(t1[:], ps_sb[:, :, 0], float(ppw[d, 0]),
                                               t0[:], AT.mult, AT.add)
                nc.scalar.activation(pd[:], t1[:], ACT.Tanh, bias=float(ppb[d]))
            for k in (range(K) if lvl >= 1 else ()):
                nc.vector.tensor_scalar(t0[:], p0[:], float(mu[k, 0]), float(isig[k, 0]),
                                        AT.subtract, AT.mult)
                nc.vector.tensor_scalar(t1[:], p1[:], float(mu[k, 1]), float(isig[k, 1]),
                                        AT.subtract, AT.mult)
                nc.vector.tensor_tensor(t0[:], t0[:], t0[:], AT.mult)
                nc.vector.tensor_tensor(t1[:], t1[:], t1[:], AT.mult)
                nc.vector.tensor_tensor(t0[:], t0[:], t1[:], AT.add)
                nc.scalar.activation(gw_sb[:, :, k], t0[:], ACT.Exp, scale=-0.5)

            # ---- projection table: tbl[n, 0:192] = (x @ w.T) as fp16 ----
            PCH = 8
            proj_iter = range(0, n_ptile, PCH) if (lvl >= 2 and not os.environ.get("MONET_NOPROJ")) else ()
            for c0 in proj_iter:
                ctiles = min(PCH, n_ptile - c0)
                r0 = c0 * 128
                nrows = min(PCH * 128, N_NODES - r0)
                lt = pj.tile([CDIM, PCH * 128], f16, name="lt", tag="lhsT")
                nc.sync.dma_start(lt[:, 0:nrows], xT[:, r0:r0 + nrows])
                cast = pj.tile([128, PCH, REAL], f16, name="cast", tag="cast")
                for t in range(ctiles):
                    tr0 = t * 128
                    ncols = min(128, nrows - tr0)
                    mmp = pp.tile([128, REAL], f32, space="PSUM", name="mmp", tag="pp", bufs=3)
                    nc.tensor.matmul(mmp[0:ncols, :], lhsT=lt[:, tr0:tr0 + ncols],
                                     rhs=w_sb[:], start=True, stop=True)
                    if t % 2 == 0:
                        nc.scalar.activation(cast[0:ncols, t, :], mmp[0:ncols, :], ACT.Copy)
                    else:
                        nc.vector.tensor_copy(cast[0:ncols, t, :], mmp[0:ncols, :])
                full = (nrows // 128) * 128
                if full:
                    nc.sync.dma_start(
                        tbl[r0:r0 + full, 0:REAL].rearrange("(t p) c -> p t c", p=128),
                        cast[:, 0:full // 128, :])
                if nrows > full:
                    rem = nrows - full
                    nc.sync.dma_start(tbl[r0 + full:r0 + nrows, 0:REAL],
                                      cast[0:rem, full // 128, :])

            # ---- message passing: two sections (lo bucket then hi bucket),
            # accumulating into SBUF so lo gathers overlap hi proj writes ----
            h_acc = io.tile([128, NBLK, OUTD], f32, name="h_acc")
            nc.vector.tensor_copy(
                h_acc[:, :, :],
                bias_sb[:, None, :].to_broadcast([128, NBLK, OUTD]))

            remaining = {(b, b_): int(tiles[b, b_]) for b in range(NBLK) for b_ in (0, 1)}
            psums = {}
            started = set()

            def evict(b, b_):
                acc = h_acc[:, b, :]
                ps = psums[(b, b_)]
                t = ev.tile([128, OUTD], f32, name="evt", tag="evt")
                nc.vector.tensor_add(t[:], ps[:, 0:OUTD], acc)
                nc.vector.tensor_add(t[:], ps[:, OUTD:2 * OUTD], t[:])
                nc.vector.tensor_add(acc, ps[:, 2 * OUTD:3 * OUTD], t[:])
                del psums[(b, b_)]

            maxg = int(os.environ.get("MONET_MAXG", "100000"))
            nq = int(os.environ.get("MONET_NQ", "4"))
            ng_done = 0
            for (b_, s0, nsl) in (gathers if lvl >= 3 else []):
                ng_done += 1
                if ng_done > maxg:
                    break
                nidx = nsl * 128
                lo = b_ * TBL_SPLIT
                hi = TBL_SPLIT if b_ == 0 else N_NODES
                g = gp.tile([128, nsl, ROW], f16, name="g", tag="g")
                nc.gpsimd.dma_gather(
                    g[:, 0:nsl, :], tbl[lo:hi, :],
                    idx_sb[:, s0 * 8:(s0 + nsl) * 8], nidx, nidx, ROW,
                    single_packet=bool(os.environ.get("MONET_SINGLEPACKET")),
                    queue_num=ng_done % nq,
                )
                if os.environ.get("MONET_GONLY"):
                    continue
                gwt = gp.tile([128, nsl, REAL], f16, name="gwt", tag="gwt")
                nc.vector.tensor_tensor(
                    out=gwt[:, 0:nsl, :].rearrange("p s (k o) -> p s k o", k=K),
                    in0=g[:, 0:nsl, 0:REAL].rearrange("p s (k o) -> p s k o", k=K),
                    in1=gw_sb[:, s0:s0 + nsl, :, None].to_broadcast([128, nsl, K, OUT_F]),
                    op=AT.mult,
                )
                oh = gp.tile([128, nsl, 128], f16, name="oh", tag="oh")
                nc.vector.tensor_tensor(
                    out=oh[:, 0:nsl, :],
                    in0=iota_sb[:, None, :].to_broadcast([128, nsl, 128]),
                    in1=dstl_sb[:, s0:s0 + nsl, None].to_broadcast([128, nsl, 128]),
                    op=AT.is_equal,
                )
                if lvl < 4:
                    continue
                for sl in range(nsl):
                    b = int(slot_blk[s0 + sl])
                    key = (b, b_)
                    if key not in psums:
                        psums[key] = pp.tile([128, REAL], f32, space="PSUM",
                                             name=f"blk{b}_{b_}", tag="blkps", bufs=5)
                    remaining[key] -= 1
                    nc.tensor.matmul(
                        psums[key][:, :], lhsT=oh[:, sl, :], rhs=gwt[:, sl, :],
                        start=(key not in started), stop=(remaining[key] == 0),
                    )
                    started.add(key)
                    if remaining[key] == 0:
                        evict(b, b_)
            if lvl >= 4:
                fullb = NPD // NB
                half = fullb // 2
                nc.sync.dma_start(
                    out[0:half * NB, :].rearrange("(t p) c -> p t c", p=128),
                    h_acc[:, 0:half, :])
                nc.sync.dma_start(
                    out[half * NB:fullb * NB, :].rearrange("(t p) c -> p t c", p=128),
                    h_acc[:, half:fullb, :])
                if NPD > fullb * NB:
                    nc.sync.dma_start(out[fullb * NB:NPD, :],
                                      h_acc[0:NPD - fullb * NB, fullb, :])

    nc.compile()
    return nc


TRACE = False           # test harness: set True to collect ntff profiles
LAST_EXEC_NS = None      # [neff1_ns, neff2_ns] after a TRACE run
LAST_RESULTS = None
LAST_PROGS = None        # [(nc1, maps1), (nc2, maps2)] for benchmarking


def kernel(feat, pseudo, edge_index,
           fc_w0, bias0, mu0, inv_sigma0, pp_w0, pp_b0,
           fc_w1, bias1, mu1, inv_sigma1, pp_w1, pp_b1):
    from concourse.bass_utils import run_bass_kernel_spmd

    feat = np.asarray(feat, np.float32)
    pseudo = np.asarray(pseudo, np.float32)
    prep = _host_prep(edge_index)
    S, gathers, slot_blk, tiles = prep["S"], prep["gathers"], prep["slot_blk"], prep["tiles"]

    # pseudo in slot layout
    psa = prep["psa"]
    psa[prep["dev_s"], prep["pos"] % 128, prep["pos"] // 128, :] = pseudo[prep["order"]]
    idxr = np.tile(prep["idx16"], (1, 8, 1))  # [NCORES, 128, S*8]

    cores = list(range(NCORES))

    scal0 = dict(ppw=np.asarray(pp_w0, np.float64), ppb=np.asarray(pp_b0, np.float64),
                 mu=np.asarray(mu0, np.float64), isig=np.asarray(inv_sigma0, np.float64))
    scal1 = dict(ppw=np.asarray(pp_w1, np.float64), ppb=np.asarray(pp_b1, np.float64),
                 mu=np.asarray(mu1, np.float64), isig=np.asarray(inv_sigma1, np.float64))

    featT = np.ascontiguousarray(feat.T).astype(np.float16)
    w0T = np.ascontiguousarray(np.asarray(fc_w0, np.float32).T).astype(np.float16)
    w1T = np.ascontiguousarray(np.asarray(fc_w1, np.float32).T).astype(np.float16)
    b0b = np.broadcast_to(np.asarray(bias0, np.float32), (128, HID)).copy()
    b1b = np.broadcast_to(np.asarray(bias1, np.float32), (128, OUT_F)).copy()

    nc1 = _build_neff(0, S, gathers, slot_blk, tiles, scal0)
    maps1 = [dict(xT=featT, wT=w0T, idx=idxr[c], dstl=prep["dstl"][c],
                  ps=psa[c], bias=b0b) for c in cores]
    res1 = run_bass_kernel_spmd(nc1, maps1, core_ids=cores, trace=TRACE)
    h = np.concatenate([res1.results[c]["out"] for c in cores], axis=0)

    hT = np.ascontiguousarray(h.T).astype(np.float16)
    nc2 = _build_neff(1, S, gathers, slot_blk, tiles, scal1)
    maps2 = [dict(xT=hT, wT=w1T, idx=idxr[c], dstl=prep["dstl"][c],
                  ps=psa[c], bias=b1b) for c in cores]
    res2 = run_bass_kernel_spmd(nc2, maps2, core_ids=cores, trace=TRACE)
    out = np.concatenate([res2.results[c]["out"] for c in cores], axis=0)
    global LAST_EXEC_NS, LAST_RESULTS, LAST_PROGS
    LAST_EXEC_NS = [res1.exec_time_ns, res2.exec_time_ns]
    LAST_RESULTS = [res1, res2]
    LAST_PROGS = [(nc1, maps1), (nc2, maps2)]
    return out

